# revision 10
# baseline (speedup 1.0000x reference)
"""DeepseekV3 decoder layer on 8 Trainium2 NeuronCores (Bass/Tile).

Software-pipelined rewrite of the baseline:
- Stage A: RMS-commute — the first RMS scale commutes through wq_a/wkv_a and
  cancels in the second RMS (eps absorbed, ~1e-6 rel effect), so the 17
  low-rank matmul chunks run on raw x and AG1a issues ~35us earlier. Only
  k_pe needs the r1 scale (64 rows).
- All RMS/softmax reciprocals: broadcast-first via ones-matmul, then a
  [128,512] DVE reciprocal (parallel across partitions) instead of a [1,512]
  one-partition reciprocal (12x faster).
- qkv: 512-col streams (block pairs), V produced token-major directly
  (ckn-tile as matmul weights), no PE transposes.
- Attention: per qc interleaved right after its q-block rope; scores/exp
  phase decoupled from the AV phase so TensorMatrix never stalls on Scalar;
  softmax denominator accumulated on Vector, reduced+broadcast in one
  all-ones f32r matmul.
- o_proj/post-LN chunks (cproj) skewed across qc iterations; MLP gate/up
  per-AG3-chunk and down-proj per-column-chunk interleaved into the tail so
  AG2/AR4/AG3/RS latencies hide under matmul.
- h2 kept in bf16 (one extra rounding of the residual stream).
"""

import numpy as np

B, S, H = 1, 2048, 2048
NH, NOPE, ROPE, VHD = 16, 128, 64, 128
QHD = NOPE + ROPE
QLR, KVLR, FF = 1536, 512, 8192
SCALE = QHD ** -0.5
EPS = 1e-6
NC = 8
SS = S // NC            # 256: sequence / output-feature shard
FFS = FF // NC          # 1024: FF shard
P = 128

TRACE = False
DEBUG = False

_CACHE = {}


def _tile_w(w):
    """[K, M] -> [K/128, ceil(M/128), 128, 128] contiguous blocks (zero-pad M)."""
    K, M = w.shape
    mc = -(-M // P)
    out = np.zeros((K // P, mc, P, P), np.float32)
    wp = np.zeros((K, mc * P), np.float32)
    wp[:, :M] = w
    for kt in range(K // P):
        for m in range(mc):
            out[kt, m] = wp[kt * P:(kt + 1) * P, m * P:(m + 1) * P]
    return out


def _build():
    if "nc" in _CACHE:
        return _CACHE["nc"]
    import concourse.mybir as mybir
    import concourse.tile as tile
    from concourse import bacc

    F32 = mybir.dt.float32
    F32R = mybir.dt.float32r
    BF16 = mybir.dt.bfloat16
    AF = mybir.ActivationFunctionType

    nc = bacc.Bacc("TRN2", target_bir_lowering=False, debug=False, num_devices=NC)

    def inp(name, shape, dt=F32):
        return nc.dram_tensor(name, list(shape), dt, kind="ExternalInput").ap()

    hT_s = inp("hT_s", [H, SS])
    hT_r = inp("hT_r", [SS, S])
    wq_a_t = inp("wq_a_t", [16, 12, P, P], BF16)
    wkv_a_t = inp("wkv_a_t", [16, 5, P, P], BF16)
    wq_b_t = inp("wq_b_t", [12, 3, P, P], BF16)
    wkv_b_t = inp("wkv_b_t", [4, 4, P, P], BF16)
    wo_t = inp("wo_t", [16, 2, P, P], BF16)
    wg_t = inp("wg_t", [16, 8, P, P], BF16)
    wu_t = inp("wu_t", [16, 8, P, P], BF16)
    wd_t = inp("wd_t", [8, 16, P, P], BF16)
    cossin = inp("cossin", [2 * P, S], BF16)        # rows 0:128 [cosT;cosT], 128:256 [sinT;sinT]
    cs_sh = inp("cs_sh", [P, SS])             # rows 0:64 cosT, 64:128 signed sinT (own shard)
    dmask = inp("dmask", [P, 4, 512], BF16)
    outT = nc.dram_tensor("outT", [SS, S], F32, kind="ExternalOutput").ap()

    RG = [list(range(NC))]

    from contextlib import ExitStack
    with tile.TileContext(nc) as tc, ExitStack() as _stack:
        cpool = _stack.enter_context(tc.tile_pool(name="const", bufs=1))
        dpool = _stack.enter_context(tc.tile_pool(name="dram", bufs=1, space="DRAM"))

        ag1a_in = dpool.tile([P, 5 * SS], BF16)
        ag1a_out = dpool.tile([NC * P, 5 * SS], BF16, addr_space="Shared")
        ag1b_in = dpool.tile([P, 12 * SS], BF16)
        ag1b_out = dpool.tile([NC * P, 12 * SS], BF16, addr_space="Shared")
        ag2_in = [dpool.tile([2 * VHD, 512], BF16, name=f"ag2_in{j}")
                  for j in range(4)]
        ag2_out = [dpool.tile([NH * VHD, 512], BF16, addr_space="Shared",
                              name=f"ag2_out{j}") for j in range(4)]
        ar4_in = [dpool.tile([1, 512], F32, name=f"ar4_in{j}") for j in range(4)]
        ar4_out = [dpool.tile([1, 512], F32, addr_space="Shared",
                              name=f"ar4_out{j}") for j in range(4)]
        ag3_in = [dpool.tile([SS, 512], BF16, name=f"ag3_in{j}") for j in range(4)]
        ag3_out = [dpool.tile([H, 512], BF16, addr_space="Shared",
                              name=f"ag3_out{j}") for j in range(4)]
        _rs_w = [512, 512, 512, 256, 256]
        rs_in = [dpool.tile([H, _rs_w[j]], BF16, name=f"rs_in{j}") for j in range(5)]
        rs_out = [dpool.tile([SS, _rs_w[j]], BF16, name=f"rs_out{j}")
                  for j in range(5)]

        ones_f = cpool.tile([P, 1], F32)
        nc.vector.memset(ones_f[:], 1.0)
        ones_r = cpool.tile([P, 1], BF16)
        nc.vector.tensor_copy(ones_r[:], ones_f[:])
        eps_t = cpool.tile([P, 1], F32)
        nc.vector.memset(eps_t[:], EPS)
        ones_k1f = cpool.tile([1, P], F32)
        nc.vector.memset(ones_k1f[:], 1.0)
        ones_k1 = cpool.tile([1, P], F32R)
        nc.vector.tensor_copy(ones_k1[:], ones_k1f[:])
        ones_ppf = cpool.tile([P, P], F32)
        nc.vector.memset(ones_ppf[:], 1.0)
        ones_pp = cpool.tile([P, P], F32R)
        nc.vector.tensor_copy(ones_pp[:], ones_ppf[:])

        # ================= Stage A: seq-shard low-rank path =================
        with tc.tile_pool(name="sa", bufs=1) as sa, \
             tc.tile_pool(name="saw", bufs=4) as saw, \
             tc.tile_pool(name="pa", bufs=2, space="PSUM") as pa:
            with nc.named_scope("stageA"):
                xs = sa.tile([P, 16, SS], F32)
                nc.sync.dma_start(xs[:], hT_s.rearrange("(kt p) s -> p kt s", p=P))
                xr = sa.tile([P, 16, SS], BF16)
                nc.vector.tensor_copy(xr[:], xs[:])
                sq = sa.tile([P, 16, SS], BF16)
                nc.vector.tensor_mul(sq[:], xs[:], xs[:])

                # kv-path matmuls on RAW x (RMS commutes; r1 only needed for kpe)
                cvs = sa.tile([P, 5, SS], F32)
                for mc in range(5):
                    wt = saw.tile([P, 16, P], BF16, tag="aw")
                    nc.sync.dma_start(wt[:], wkv_a_t[:, mc].rearrange("a p m -> p a m"))
                    ps = pa.tile([P, SS], F32, tag="amm")
                    for kt in range(16):
                        nc.tensor.matmul(ps[:], wt[:, kt], xr[:, kt],
                                         start=(kt == 0), stop=(kt == 15))
                    nc.vector.tensor_copy(cvs[:, mc], ps[:])

                # r1 (for k_pe only): sum(x^2) -> bcast 64 -> sqrt -> recip
                msq_ps = pa.tile([1, SS], F32, tag="msq")
                for kt in range(16):
                    nc.tensor.matmul(msq_ps[:], ones_r[:], sq[:, kt],
                                     start=(kt == 0), stop=(kt == 15))
                msq_r = sa.tile([1, SS], F32R)
                with nc.allow_low_precision(reason="f32r copy of rms stats"):
                    nc.vector.tensor_copy(msq_r[:], msq_ps[:])
                b1_ps = pa.tile([64, SS], F32, tag="rb")
                nc.tensor.matmul(b1_ps[:], ones_k1[:, :64], msq_r[:],
                                 start=True, stop=True)
                r1s = sa.tile([64, SS], F32)
                nc.scalar.activation(r1s[:], b1_ps[:], AF.Sqrt, scale=1.0 / H,
                                     bias=eps_t[:64])
                r1b = sa.tile([64, SS], F32)
                nc.vector.reciprocal(r1b[:], r1s[:])

                # kv RMS on raw cv (r1 cancels; eps absorbed)
                sq3 = sa.tile([P, 4, SS], BF16)
                nc.vector.tensor_mul(sq3[:], cvs[:, :4], cvs[:, :4])
                msq3 = pa.tile([1, SS], F32, tag="msq")
                for mc in range(4):
                    nc.tensor.matmul(msq3[:], ones_r[:], sq3[:, mc],
                                     start=(mc == 0), stop=(mc == 3))
                msq3_r = sa.tile([1, SS], F32R)
                with nc.allow_low_precision(reason="f32r copy of rms stats"):
                    nc.vector.tensor_copy(msq3_r[:], msq3[:])
                b3_ps = pa.tile([P, SS], F32, tag="rb")
                nc.tensor.matmul(b3_ps[:], ones_k1[:], msq3_r[:],
                                 start=True, stop=True)
                r3s = sa.tile([P, SS], F32)
                nc.scalar.activation(r3s[:], b3_ps[:], AF.Sqrt, scale=1.0 / KVLR,
                                     bias=eps_t[:])
                r3b = sa.tile([P, SS], F32)
                nc.vector.reciprocal(r3b[:], r3s[:])
                ckn = sa.tile([P, 4, SS], BF16)
                nc.vector.tensor_mul(ckn[:], cvs[:, :4],
                                     r3b[:, None, :].to_broadcast([P, 4, SS]))

                # k_pe rope on cvs[:64, 4] * r1 (cs_sh rows 0:64 cos, 64:128 signed sin)
                cos_sh = sa.tile([64, SS], F32)
                nc.sync.dma_start(cos_sh[:], cs_sh[0:64, :])
                sin_sh = sa.tile([64, SS], F32)
                nc.sync.dma_start(sin_sh[:], cs_sh[64:128, :])
                ksw = sa.tile([64, SS], F32)
                nc.sync.dma_start(ksw[0:32, :], cvs[32:64, 4])
                nc.sync.dma_start(ksw[32:64, :], cvs[0:32, 4])
                kpe_c = sa.tile([64, SS], F32)
                nc.vector.tensor_mul(kpe_c[:], cvs[:64, 4], cos_sh[:])
                t1 = sa.tile([64, SS], F32)
                nc.vector.tensor_mul(t1[:], ksw[:], sin_sh[:])
                nc.vector.tensor_add(kpe_c[:], kpe_c[:], t1[:])
                kpe_n = sa.tile([64, SS], BF16)
                nc.vector.tensor_mul(kpe_n[:], kpe_c[:], r1b[:])

                nc.sync.dma_start(
                    ag1a_in[:, 0:4 * SS].rearrange("p (kt s) -> p kt s", s=SS),
                    ckn[:])
                nc.sync.dma_start(ag1a_in[:64, 4 * SS:5 * SS], kpe_n[:])
                nc.gpsimd.collective_compute(
                    "AllGather", mybir.AluOpType.bypass, replica_groups=RG,
                    ins=[ag1a_in], outs=[ag1a_out])

                # q-path on RAW x
                us = sa.tile([P, 12, SS], F32)
                for mc in range(12):
                    wt = saw.tile([P, 16, P], BF16, tag="aw")
                    nc.sync.dma_start(wt[:], wq_a_t[:, mc].rearrange("a p m -> p a m"))
                    ps = pa.tile([P, SS], F32, tag="amm")
                    for kt in range(16):
                        nc.tensor.matmul(ps[:], wt[:, kt], xr[:, kt],
                                         start=(kt == 0), stop=(kt == 15))
                    nc.vector.tensor_copy(us[:, mc], ps[:])

                sq2 = sa.tile([P, 12, SS], BF16)
                nc.vector.tensor_mul(sq2[:], us[:], us[:])
                msq2 = pa.tile([1, SS], F32, tag="msq")
                for mc in range(12):
                    nc.tensor.matmul(msq2[:], ones_r[:], sq2[:, mc],
                                     start=(mc == 0), stop=(mc == 11))
                msq2_r = sa.tile([1, SS], F32R)
                with nc.allow_low_precision(reason="f32r copy of rms stats"):
                    nc.vector.tensor_copy(msq2_r[:], msq2[:])
                b2_ps = pa.tile([P, SS], F32, tag="rb")
                nc.tensor.matmul(b2_ps[:], ones_k1[:], msq2_r[:],
                                 start=True, stop=True)
                r2s = sa.tile([P, SS], F32)
                nc.scalar.activation(r2s[:], b2_ps[:], AF.Sqrt, scale=1.0 / QLR,
                                     bias=eps_t[:])
                r2b = sa.tile([P, SS], F32)
                nc.vector.reciprocal(r2b[:], r2s[:])
                un = sa.tile([P, 12, SS], BF16)
                nc.vector.tensor_mul(un[:], us[:],
                                     r2b[:, None, :].to_broadcast([P, 12, SS]))
                nc.sync.dma_start(
                    ag1b_in.rearrange("p (kt s) -> p kt s", s=SS), un[:])
                nc.gpsimd.collective_compute(
                    "AllGather", mybir.AluOpType.bypass, replica_groups=RG,
                    ins=[ag1b_in], outs=[ag1b_out])

        # ========== Stage B: qkv + attention + cproj + MLP, interleaved ======
        # Outer pools (whole stage B): h2/residual, cproj tiles, gate/up h0.
        with tc.tile_pool(name="sh2", bufs=1) as sh2, \
             tc.tile_pool(name="scp", bufs=1) as scp, \
             tc.tile_pool(name="sgu", bufs=1) as sgu, \
             tc.tile_pool(name="pcp", bufs=1, space="PSUM") as pcp:

            h2 = sh2.tile([P, 2, S], BF16)
            wos = scp.tile([P, 16, 2, P], BF16)
            nc.sync.dma_start(wos[:], wo_t.rearrange("a b p m -> p a b m"))
            wg0 = sgu.tile([P, 16, 4, P], BF16, name="wg0")
            wu0 = sgu.tile([P, 16, 4, P], BF16, name="wu0")
            for m in range(4):
                nc.sync.dma_start(wg0[:, :, m, :],
                                  wg_t[:, m].rearrange("a p m -> p a m"))
                nc.sync.dma_start(wu0[:, :, m, :],
                                  wu_t[:, m].rearrange("a p m -> p a m"))

            def cproj_a(j):
                """o_proj chunk j + residual + sq-stats + AR4_j."""
                nsl = slice(j * 512, (j + 1) * 512)
                resid = scp.tile([P, 2, 512], F32, tag="resid", bufs=1,
                                 name="resid")
                nc.sync.dma_start(
                    resid[:],
                    hT_r.rearrange("(mc p) s -> p mc s", p=P)[:, :, nsl])
                pss = [pcp.tile([P, 512], F32, tag="omm", bufs=2, name="omm")
                       for _ in range(2)]
                for half in range(2):
                    rhs = scp.tile([P, 8, 512], BF16, tag="rhs2", bufs=2,
                                   name="rhs2")
                    nc.sync.dma_start(
                        rhs[:],
                        ag2_out[j][half * 8 * P:(half + 1) * 8 * P, :].rearrange(
                            "(kt p) s -> p kt s", p=P))
                    for mc in range(2):
                        for ktl in range(8):
                            nc.tensor.matmul(
                                pss[mc][:], wos[:, half * 8 + ktl, mc],
                                rhs[:, ktl],
                                start=(half == 0 and ktl == 0),
                                stop=(half == 1 and ktl == 7))
                sqh = scp.tile([P, 2, 512], BF16, tag="sqh", bufs=1, name="sqh")
                for mc in range(2):
                    nc.vector.tensor_add(h2[:, mc, nsl], pss[mc][:], resid[:, mc])
                    nc.vector.tensor_mul(sqh[:, mc], h2[:, mc, nsl],
                                         h2[:, mc, nsl])
                mt = pcp.tile([P, 512], F32, tag="misc", bufs=1, name="m4")
                for mc in range(2):
                    nc.tensor.matmul(mt[0:1, :], ones_r[:], sqh[:, mc],
                                     start=(mc == 0), stop=(mc == 1))
                msq4 = scp.tile([1, 512], F32, tag="msq4", bufs=1, name="msq4")
                nc.vector.tensor_copy(msq4[:], mt[0:1, :])
                nc.sync.dma_start(ar4_in[j][:, :], msq4[:])
                nc.gpsimd.collective_compute(
                    "AllReduce", mybir.AluOpType.add, replica_groups=RG,
                    ins=[ar4_in[j]], outs=[ar4_out[j]])

            def cproj_b(j):
                """r4_j + yT_j + AG3_j."""
                nsl = slice(j * 512, (j + 1) * 512)
                msq4g = scp.tile([1, 512], F32, tag="m4g", bufs=1, name="msq4g")
                nc.sync.dma_start(msq4g[:], ar4_out[j][:, :])
                msq4r = scp.tile([1, 512], F32R, tag="m4r", bufs=1, name="msq4r")
                with nc.allow_low_precision(reason="f32r copy of rms stats"):
                    nc.vector.tensor_copy(msq4r[:], msq4g[:])
                b4_ps = pcp.tile([P, 512], F32, tag="misc", bufs=1, name="b4")
                nc.tensor.matmul(b4_ps[:], ones_k1[:], msq4r[:],
                                 start=True, stop=True)
                r4s = scp.tile([P, 512], F32, tag="r4s", bufs=1, name="r4s")
                nc.scalar.activation(r4s[:], b4_ps[:], AF.Sqrt,
                                     scale=1.0 / H, bias=eps_t[:])
                r4b = scp.tile([P, 512], F32, tag="r4b", bufs=1, name="r4b")
                nc.vector.reciprocal(r4b[:], r4s[:])
                yT = scp.tile([P, 2, 512], BF16, tag="yT", bufs=1, name="yT")
                nc.vector.tensor_mul(
                    yT[:], h2[:, :, nsl],
                    r4b[:, None, :].to_broadcast([P, 2, 512]))
                nc.sync.dma_start(
                    ag3_in[j].rearrange("(mc p) s -> p mc s", p=P), yT[:])
                nc.gpsimd.collective_compute(
                    "AllGather", mybir.AluOpType.bypass, replica_groups=RG,
                    ins=[ag3_in[j]], outs=[ag3_out[j]])

            # ---- inner phase: qkv + attention (pools released before tail) --
            with tc.tile_pool(name="satt", bufs=1) as satt, \
                 tc.tile_pool(name="qph", bufs=1) as qph, \
                 tc.tile_pool(name="sbr", bufs=2) as sbr, \
                 tc.tile_pool(name="sbe", bufs=2) as sbe, \
                 tc.tile_pool(name="pq", bufs=1, space="PSUM") as pq, \
                 tc.tile_pool(name="pat", bufs=1, space="PSUM") as pat:

                qT = satt.tile([P, 2, S], BF16)
                kT = satt.tile([P, 2, S], BF16)
                kpeT = satt.tile([64, S], BF16)
                v_tok = satt.tile([P, 32 * P], BF16)
                qrot = [satt.tile([64, 2, 512], BF16, name=f"qrot{i}")
                        for i in range(4)]
                es_all = satt.tile([P, 16, 512], BF16)
                dacc = satt.tile([P, 512], F32R)
                mask_t = satt.tile([P, 4, 512], BF16)
                nc.sync.dma_start(mask_t[:], dmask[:, :, :])
                cos_t = qph.tile([P, S], BF16)
                nc.sync.dma_start(cos_t[:], cossin[0:P, :])
                sin_t = qph.tile([P, S], BF16)
                nc.sync.dma_start(sin_t[:], cossin[P:2 * P, :])
                wkb = qph.tile([P, 4, 4, P], BF16)
                nc.sync.dma_start(wkb[:], wkv_b_t.rearrange("a b p m -> p a b m"))
                wqb = qph.tile([P, 12, 3, P], BF16)
                nc.sync.dma_start(wqb[:], wq_b_t.rearrange("a b p m -> p a b m"))

                def attn(qc):
                    """Attention for query chunk qc, both heads; oT + AG2."""
                    qsl = slice(qc * 512, (qc + 1) * 512)
                    nkt = 4 * qc + 4
                    oT = scp.tile([P, 2, 512], BF16, tag="oT", bufs=1,
                                  name="oT")
                    for h in range(2):
                        for kt in range(nkt):
                            ksl = slice(kt * P, (kt + 1) * P)
                            sc_ps = pat.tile([P, 512], F32, tag="sc", bufs=2,
                                             name="scp")
                            nc.tensor.matmul(sc_ps[:], kT[:, h, ksl],
                                             qT[:, h, qsl],
                                             start=True, stop=False)
                            nc.tensor.matmul(sc_ps[:], kpeT[:, ksl],
                                             qrot[qc][:, h, :],
                                             start=False, stop=True)
                            j = kt - 4 * qc
                            if j >= 0:
                                nc.vector.tensor_add(sc_ps[:], sc_ps[:],
                                                     mask_t[:, j])
                            nc.scalar.activation(es_all[:, kt], sc_ps[:], AF.Exp)
                            with nc.allow_low_precision(reason="f32r denom"):
                                if kt == 0:
                                    nc.vector.tensor_copy(dacc[:],
                                                          es_all[:, kt])
                                else:
                                    nc.vector.tensor_add(dacc[:], dacc[:],
                                                         es_all[:, kt])
                        o_ps = pat.tile([P, 512], F32, tag="o", bufs=1,
                                        name="o")
                        for kt in range(nkt):
                            nc.tensor.matmul(
                                o_ps[:],
                                v_tok[:, (h * 16 + kt) * P:(h * 16 + kt + 1) * P],
                                es_all[:, kt],
                                start=(kt == 0), stop=(kt == nkt - 1))
                        rb_ps = pcp.tile([P, 512], F32, tag="misc", bufs=1,
                                         name="rbo")
                        nc.tensor.matmul(rb_ps[:], ones_pp[:], dacc[:],
                                         start=True, stop=True)
                        recb = sbe.tile([P, 512], F32, tag="recb", bufs=1)
                        nc.vector.reciprocal(recb[:], rb_ps[:])
                        nc.vector.tensor_mul(oT[:, h], o_ps[:], recb[:])
                    nc.sync.dma_start(
                        ag2_in[qc].rearrange("(mc p) s -> p mc s", p=P), oT[:])
                    nc.gpsimd.collective_compute(
                        "AllGather", mybir.AluOpType.bypass, replica_groups=RG,
                        ins=[ag2_in[qc]], outs=[ag2_out[qc]])

                # ---- kv expansion over 4 block-pairs (512 cols each) ----
                with nc.named_scope("stageB_kv"):
                    for bp in range(4):
                        sl = slice(bp * 512, (bp + 1) * 512)
                        rhs_c = sbr.tile([P, 4, 512], BF16, tag="rhs1c",
                                         bufs=2)
                        for half in range(2):
                            blk = 2 * bp + half
                            nc.sync.dma_start(
                                rhs_c[:, :, half * SS:(half + 1) * SS],
                                ag1a_out[blk * P:(blk + 1) * P,
                                         0:4 * SS].rearrange(
                                    "p (kt s) -> p kt s", s=SS))
                            nc.sync.dma_start(
                                kpeT[:, blk * SS:(blk + 1) * SS],
                                ag1a_out[blk * P:blk * P + 64, 4 * SS:5 * SS])
                        for mc in range(2):
                            ps = pq.tile([P, 512], F32, tag="qb", bufs=2,
                                         name="qbp")
                            for kt in range(4):
                                nc.tensor.matmul(ps[:], wkb[:, kt, mc],
                                                 rhs_c[:, kt],
                                                 start=(kt == 0),
                                                 stop=(kt == 3))
                            nc.vector.tensor_copy(kT[:, mc, sl], ps[:])
                        for tt in range(4):
                            tsl = slice(tt * P, (tt + 1) * P)
                            vps = pq.tile([P, 512], F32, tag="qb", bufs=2,
                                          name="qbp")
                            for kt in range(4):
                                nc.tensor.matmul(vps[:, 0:2 * P],
                                                 rhs_c[:, kt, tsl],
                                                 wkb[:, kt, 2:4, :],
                                                 start=(kt == 0),
                                                 stop=(kt == 3))
                            ttg = 4 * bp + tt
                            nc.vector.tensor_copy(
                                v_tok[:, ttg * P:(ttg + 1) * P], vps[:, 0:P])
                            nc.vector.tensor_copy(
                                v_tok[:, (16 + ttg) * P:(17 + ttg) * P],
                                vps[:, P:2 * P])

                # ---- q expansion + rope per block-pair, attn interleaved ----
                with nc.named_scope("stageB_qa"):
                    for bp in range(4):
                        sl = slice(bp * 512, (bp + 1) * 512)
                        rhs_u = sbr.tile([P, 12, 512], BF16, tag="rhs1",
                                         bufs=2)
                        for half in range(2):
                            blk = 2 * bp + half
                            nc.sync.dma_start(
                                rhs_u[:, :, half * SS:(half + 1) * SS],
                                ag1b_out[blk * P:(blk + 1) * P, :].rearrange(
                                    "p (kt s) -> p kt s", s=SS))
                        for mc in range(2):
                            ps = pq.tile([P, 512], F32, tag="qb", bufs=2,
                                         name="qbp")
                            for kt in range(12):
                                nc.tensor.matmul(ps[:], wqb[:, kt, mc],
                                                 rhs_u[:, kt],
                                                 start=(kt == 0),
                                                 stop=(kt == 11))
                            nc.vector.tensor_copy(qT[:, mc, sl], ps[:])
                        qpr_ps = pq.tile([P, 512], F32, tag="qb", bufs=2,
                                         name="qbp")
                        for kt in range(12):
                            nc.tensor.matmul(qpr_ps[:], wqb[:, kt, 2],
                                             rhs_u[:, kt],
                                             start=(kt == 0), stop=(kt == 11))
                        qpr = sbe.tile([P, 512], BF16, tag="qpr", bufs=1)
                        nc.vector.tensor_copy(qpr[:], qpr_ps[:])
                        qsw = sbe.tile([P, 512], BF16, tag="qsw", bufs=1)
                        for b in (0, 64):
                            nc.sync.dma_start(qsw[b:b + 32, :],
                                              qpr[b + 32:b + 64, :])
                            nc.sync.dma_start(qsw[b + 32:b + 64, :],
                                              qpr[b:b + 32, :])
                        qc1 = sbe.tile([P, 512], BF16, tag="qc1", bufs=1)
                        nc.vector.tensor_mul(qc1[:], qpr[:], cos_t[:, sl])
                        qs1 = sbe.tile([P, 512], BF16, tag="qs1", bufs=1)
                        nc.vector.tensor_mul(qs1[:], qsw[:], sin_t[:, sl])
                        qro = sbe.tile([P, 512], BF16, tag="qro", bufs=1)
                        nc.vector.tensor_add(qro[:], qc1[:], qs1[:])
                        nc.sync.dma_start(qrot[bp][:, 0, :], qro[0:64, :])
                        nc.sync.dma_start(qrot[bp][:, 1, :], qro[64:128, :])

                        with nc.named_scope("stageB_attn"):
                            attn(bp)
                            if bp >= 1:
                                cproj_a(bp - 1)
                            if bp >= 2:
                                cproj_b(bp - 2)

            # ---- tail: remaining cproj + MLP fully interleaved ----
            with tc.tile_pool(name="smlp", bufs=1) as smlp, \
                 tc.tile_pool(name="pml", bufs=1, space="PSUM") as pml:
                with nc.named_scope("stageD"):
                    wg1 = smlp.tile([P, 16, 4, P], BF16, name="wg1")
                    wu1 = smlp.tile([P, 16, 4, P], BF16, name="wu1")
                    for m in range(4):
                        nc.sync.dma_start(
                            wg1[:, :, m, :],
                            wg_t[:, 4 + m].rearrange("a p m -> p a m"))
                        nc.sync.dma_start(
                            wu1[:, :, m, :],
                            wu_t[:, 4 + m].rearrange("a p m -> p a m"))
                    wds = smlp.tile([P, 8, 16, P], BF16, name="wds")
                    nc.sync.dma_start(wds[:], wd_t.rearrange("a b p m -> p a b m"))

                    acts = {}

                    def gu(half, ncol, wg_s, wu_s):
                        """gate/up for weight half `half`, seq chunk ncol."""
                        nsl = slice(ncol * 512, (ncol + 1) * 512)
                        if ncol not in acts:
                            acts[ncol] = smlp.tile([P, 8, 512], BF16,
                                                   tag="act", bufs=2,
                                                   name=f"act{ncol}")
                        atile = acts[ncol]
                        rhs = smlp.tile([P, 16, 512], BF16, tag="rhs3", bufs=2)
                        nc.sync.dma_start(
                            rhs[:],
                            ag3_out[ncol].rearrange("(kt p) s -> p kt s", p=P))
                        for m in range(4):
                            gp = pml.tile([P, 512], F32, tag="g", bufs=1,
                                          name="gps")
                            up = pml.tile([P, 512], F32, tag="u", bufs=1,
                                          name="ups")
                            for kt in range(16):
                                nc.tensor.matmul(gp[:], wg_s[:, kt, m],
                                                 rhs[:, kt],
                                                 start=(kt == 0),
                                                 stop=(kt == 15))
                                nc.tensor.matmul(up[:], wu_s[:, kt, m],
                                                 rhs[:, kt],
                                                 start=(kt == 0),
                                                 stop=(kt == 15))
                            gsil = smlp.tile([P, 512], BF16, tag="gsil",
                                             bufs=1)
                            nc.scalar.activation(gsil[:], gp[:], AF.Silu)
                            nc.vector.tensor_mul(atile[:, half * 4 + m, :],
                                                 gsil[:], up[:])

                    CH = [(0, 512), (512, 512), (1024, 512), (1536, 256),
                          (1792, 256)]

                    def down(j):
                        """down-proj partials for col chunk j + ReduceScatter."""
                        c0, cw = CH[j]
                        ncol = c0 // 512
                        off = c0 - ncol * 512
                        atile = acts[ncol]
                        for mc in range(16):
                            ps = pml.tile([P, 512], F32, tag="dmm", bufs=2,
                                          name="dmmps")[:, :cw]
                            for kt in range(8):
                                nc.tensor.matmul(
                                    ps[:], wds[:, kt, mc],
                                    atile[:, kt, off:off + cw],
                                    start=(kt == 0), stop=(kt == 7))
                            dn = smlp.tile([P, 512], BF16, tag="dn", bufs=2,
                                           name="dntile")[:, :cw]
                            if mc % 2 == 0:
                                nc.vector.tensor_copy(dn[:], ps[:])
                            else:
                                nc.scalar.activation(dn[:], ps[:], AF.Copy)
                            nc.sync.dma_start(rs_in[j][mc * P:(mc + 1) * P, :],
                                              dn[:])
                        nc.gpsimd.collective_compute(
                            "ReduceScatter", mybir.AluOpType.add,
                            replica_groups=RG, ins=[rs_in[j]], outs=[rs_out[j]])

                    def fin(j):
                        """RS_j output + residual -> outT columns."""
                        c0, cw = CH[j]
                        nsl = slice(c0, c0 + cw)
                        ft = smlp.tile([P, 2, 512], BF16, tag="fin", bufs=1,
                                       name="fintile")[:, :, :cw]
                        nc.sync.dma_start(
                            ft[:],
                            rs_out[j].rearrange("(mc p) s -> p mc s", p=P))
                        fo = smlp.tile([P, 2, 512], F32, tag="fino", bufs=1,
                                       name="finotile")[:, :, :cw]
                        nc.vector.tensor_add(fo[:], ft[:], h2[:, :, nsl])
                        nc.sync.dma_start(
                            outT.rearrange("(mc p) s -> p mc s", p=P)[:, :, nsl],
                            fo[:])

                    gu(0, 0, wg0, wu0)
                    cproj_a(3)
                    gu(0, 1, wg0, wu0)
                    cproj_b(2)
                    gu(1, 0, wg1, wu1)
                    down(0)
                    gu(0, 2, wg0, wu0)
                    cproj_b(3)
                    gu(1, 1, wg1, wu1)
                    down(1)
                    fin(0)
                    gu(0, 3, wg0, wu0)
                    gu(1, 2, wg1, wu1)
                    down(2)
                    fin(1)
                    gu(1, 3, wg1, wu1)
                    down(3)
                    fin(2)
                    down(4)
                    fin(3)
                    fin(4)

    nc.compile()
    _CACHE["nc"] = nc
    return nc


def _host_prep(inputs):
    import ml_dtypes
    bf16 = ml_dtypes.bfloat16
    inp = {k: np.asarray(v) for k, v in inputs.items()}
    hidden = inp["hidden_states"].reshape(S, H).astype(np.float32)
    pos = inp["position_ids"].reshape(S).astype(np.int64)
    cosT = inp["cos"][pos].T.astype(np.float32)
    sinT = inp["sin"][pos].T.astype(np.float32)
    wq_a = (inp["wq_a"] * inp["in_ln"][:, None]).astype(np.float32)
    wkv_a = (inp["wkv_a"] * inp["in_ln"][:, None]).astype(np.float32)
    wq_b = (inp["wq_b"] * inp["q_a_ln"][:, None]).astype(np.float32)
    wkv_b = (inp["wkv_b"] * inp["kv_a_ln"][:, None]).astype(np.float32)
    wg = (inp["w_gate"] * inp["post_ln"][:, None]).astype(np.float32)
    wu = (inp["w_up"] * inp["post_ln"][:, None]).astype(np.float32)
    wd = inp["w_down"].astype(np.float32)
    wo = inp["wo"].astype(np.float32)

    de = np.empty(ROPE, np.int64)
    de[:32] = np.arange(32) * 2
    de[32:] = np.arange(32) * 2 + 1
    wkv_a = np.concatenate([wkv_a[:, :KVLR], wkv_a[:, KVLR:][:, de]], axis=1)
    wq_b = wq_b.reshape(QLR, NH, QHD)
    wkv_b = wkv_b.reshape(KVLR, NH, NOPE + VHD)

    hT = hidden.T.copy()
    sin_sg = np.concatenate([-sinT[:32], sinT[32:]], axis=0)    # signed for swap trick
    cossin = np.concatenate([cosT, cosT, sin_sg, sin_sg], axis=0)  # (256, S)
    ki = np.arange(P)[:, None]
    qi = np.arange(512)[None, :]
    dmask = np.stack([np.where(qi >= j * P + ki, 0.0, -1e30).astype(np.float32)
                      for j in range(4)], axis=1)               # (128, 4, 512)

    wq_a_t = _tile_w(wq_a)
    wkv_a_t = _tile_w(wkv_a)

    in_maps = []
    for c in range(NC):
        h0, h1 = 2 * c, 2 * c + 1
        qb = np.concatenate([
            wq_b[:, h0, :NOPE], wq_b[:, h1, :NOPE],
            wq_b[:, h0, NOPE:][:, de], wq_b[:, h1, NOPE:][:, de]], axis=1) * SCALE
        kb = np.concatenate([
            wkv_b[:, h0, :NOPE], wkv_b[:, h1, :NOPE],
            wkv_b[:, h0, NOPE:], wkv_b[:, h1, NOPE:]], axis=1)
        ssl = slice(c * SS, (c + 1) * SS)
        cs_sh = np.concatenate([cosT[:, ssl], sin_sg[:, ssl]], axis=0)
        in_maps.append({
            "hT_s": np.ascontiguousarray(hT[:, ssl]),
            "hT_r": np.ascontiguousarray(hT[ssl, :]),
            "wq_a_t": wq_a_t.astype(bf16),
            "wkv_a_t": wkv_a_t.astype(bf16),
            "wq_b_t": _tile_w(qb.astype(np.float32)).astype(bf16),
            "wkv_b_t": _tile_w(kb.astype(np.float32)).astype(bf16),
            "wo_t": _tile_w(np.ascontiguousarray(wo[:, ssl])).astype(bf16),
            "wg_t": _tile_w(wg[:, c * FFS:(c + 1) * FFS]).astype(bf16),
            "wu_t": _tile_w(wu[:, c * FFS:(c + 1) * FFS]).astype(bf16),
            "wd_t": _tile_w(wd[c * FFS:(c + 1) * FFS, :]).astype(bf16),
            "cossin": cossin.astype(bf16),
            "cs_sh": np.ascontiguousarray(cs_sh),
            "dmask": dmask.astype(bf16),
        })
    return in_maps


_LAST_RESULT = {}


def kernel(**inputs) -> np.ndarray:
    from concourse.bass_utils import run_bass_kernel_spmd
    nc = _build()
    in_maps = _host_prep(inputs)
    kwargs = {}
    if TRACE:
        import sys, types
        if "antenv.axon_hooks" not in sys.modules:
            try:
                from trn_agent_boot.trn_boot import _ntff_profile_via_ctypes
                mod = types.ModuleType("antenv.axon_hooks")
                _hook = _ntff_profile_via_ctypes('/opt/axon/libaxon_pjrt.so')
                mod.get_axon_ntff_profile_hook = lambda: _hook
                mod.set_axon_ntff_profile_hook = lambda h: None
                sys.modules["antenv.axon_hooks"] = mod
                import antenv
                antenv.axon_hooks = mod
            except Exception:
                pass
        kwargs["trace"] = True
    res = run_bass_kernel_spmd(nc, in_maps, list(range(NC)), **kwargs)
    _LAST_RESULT["res"] = res
    outT = np.concatenate([res.results[c]["outT"] for c in range(NC)], axis=0)
    return np.ascontiguousarray(outT.T)[None].astype(np.float32)


# revision 13
# speedup vs baseline: 1.0264x; 1.0264x over previous
"""DeepseekV3 decoder layer on 8 Trainium2 NeuronCores (Bass/Tile).

Software-pipelined rewrite of the baseline:
- Stage A: RMS-commute — the first RMS scale commutes through wq_a/wkv_a and
  cancels in the second RMS (eps absorbed, ~1e-6 rel effect), so the 17
  low-rank matmul chunks run on raw x and AG1a issues ~35us earlier. Only
  k_pe needs the r1 scale (64 rows).
- All RMS/softmax reciprocals: broadcast-first via ones-matmul, then a
  [128,512] DVE reciprocal (parallel across partitions) instead of a [1,512]
  one-partition reciprocal (12x faster).
- qkv: 512-col streams (block pairs), V produced token-major directly
  (ckn-tile as matmul weights), no PE transposes.
- Attention: per qc interleaved right after its q-block rope; scores/exp
  phase decoupled from the AV phase so TensorMatrix never stalls on Scalar;
  softmax denominator accumulated on Vector, reduced+broadcast in one
  all-ones f32r matmul.
- o_proj/post-LN chunks (cproj) skewed across qc iterations; MLP gate/up
  per-AG3-chunk and down-proj per-column-chunk interleaved into the tail so
  AG2/AR4/AG3/RS latencies hide under matmul.
- h2 kept in bf16 (one extra rounding of the residual stream).
"""

import numpy as np

B, S, H = 1, 2048, 2048
NH, NOPE, ROPE, VHD = 16, 128, 64, 128
QHD = NOPE + ROPE
QLR, KVLR, FF = 1536, 512, 8192
SCALE = QHD ** -0.5
EPS = 1e-6
NC = 8
SS = S // NC            # 256: sequence / output-feature shard
FFS = FF // NC          # 1024: FF shard
P = 128

TRACE = False
DEBUG = False

_CACHE = {}


def _tile_w(w):
    """[K, M] -> [K/128, ceil(M/128), 128, 128] contiguous blocks (zero-pad M)."""
    K, M = w.shape
    mc = -(-M // P)
    out = np.zeros((K // P, mc, P, P), np.float32)
    wp = np.zeros((K, mc * P), np.float32)
    wp[:, :M] = w
    for kt in range(K // P):
        for m in range(mc):
            out[kt, m] = wp[kt * P:(kt + 1) * P, m * P:(m + 1) * P]
    return out


def _build():
    if "nc" in _CACHE:
        return _CACHE["nc"]
    import concourse.mybir as mybir
    import concourse.tile as tile
    from concourse import bacc

    F32 = mybir.dt.float32
    F32R = mybir.dt.float32r
    BF16 = mybir.dt.bfloat16
    AF = mybir.ActivationFunctionType

    nc = bacc.Bacc("TRN2", target_bir_lowering=False, debug=False, num_devices=NC)

    def inp(name, shape, dt=F32):
        return nc.dram_tensor(name, list(shape), dt, kind="ExternalInput").ap()

    hT_s = inp("hT_s", [H, SS])
    hT_r = inp("hT_r", [SS, S])
    wq_a_t = inp("wq_a_t", [P, 16, 12, P], BF16)
    wkv_a_t = inp("wkv_a_t", [P, 16, 5, P], BF16)
    wq_b_t = inp("wq_b_t", [P, 12, 3, P], BF16)
    wkv_b_t = inp("wkv_b_t", [P, 4, 4, P], BF16)
    wo_t = inp("wo_t", [P, 16, 2, P], BF16)
    wg_t = inp("wg_t", [P, 2, 16, 4, P], BF16)
    wu_t = inp("wu_t", [P, 2, 16, 4, P], BF16)
    wd_t = inp("wd_t", [P, 8, 16, P], BF16)
    cossin = inp("cossin", [2 * P, S], BF16)        # rows 0:128 [cosT;cosT], 128:256 [sinT;sinT]
    cs_sh = inp("cs_sh", [P, SS])             # rows 0:64 cosT, 64:128 signed sinT (own shard)
    dmask = inp("dmask", [P, 4, 512], BF16)
    outT = nc.dram_tensor("outT", [SS, S], F32, kind="ExternalOutput").ap()

    RG = [list(range(NC))]

    from contextlib import ExitStack
    with tile.TileContext(nc) as tc, ExitStack() as _stack:
        cpool = _stack.enter_context(tc.tile_pool(name="const", bufs=1))
        dpool = _stack.enter_context(tc.tile_pool(name="dram", bufs=1, space="DRAM"))

        ag1a_in = dpool.tile([P, 5 * SS], BF16)
        ag1a_out = dpool.tile([NC * P, 5 * SS], BF16, addr_space="Shared")
        ag1b_in = dpool.tile([P, 12 * SS], BF16)
        ag1b_out = dpool.tile([NC * P, 12 * SS], BF16, addr_space="Shared")
        ag2_in = [dpool.tile([2 * VHD, 512], BF16, name=f"ag2_in{j}")
                  for j in range(4)]
        ag2_out = [dpool.tile([NH * VHD, 512], BF16, addr_space="Shared",
                              name=f"ag2_out{j}") for j in range(4)]
        ar4_in = [dpool.tile([1, 512], F32, name=f"ar4_in{j}") for j in range(4)]
        ar4_out = [dpool.tile([1, 512], F32, addr_space="Shared",
                              name=f"ar4_out{j}") for j in range(4)]
        ag3_in = [dpool.tile([SS, 512], BF16, name=f"ag3_in{j}") for j in range(4)]
        ag3_out = [dpool.tile([H, 512], BF16, addr_space="Shared",
                              name=f"ag3_out{j}") for j in range(4)]
        _rs_w = [512, 512, 512, 256, 256]
        rs_in = [dpool.tile([H, _rs_w[j]], BF16, name=f"rs_in{j}") for j in range(5)]
        rs_out = [dpool.tile([SS, _rs_w[j]], BF16, name=f"rs_out{j}")
                  for j in range(5)]

        ones_f = cpool.tile([P, 1], F32)
        nc.vector.memset(ones_f[:], 1.0)
        ones_r = cpool.tile([P, 1], BF16)
        nc.vector.tensor_copy(ones_r[:], ones_f[:])
        eps_t = cpool.tile([P, 1], F32)
        nc.vector.memset(eps_t[:], EPS)
        ones_k1f = cpool.tile([1, P], F32)
        nc.vector.memset(ones_k1f[:], 1.0)
        ones_k1 = cpool.tile([1, P], F32R)
        nc.vector.tensor_copy(ones_k1[:], ones_k1f[:])
        ones_ppf = cpool.tile([P, P], F32)
        nc.vector.memset(ones_ppf[:], 1.0)
        ones_pp = cpool.tile([P, P], F32R)
        nc.vector.tensor_copy(ones_pp[:], ones_ppf[:])

        # ================= Stage A: seq-shard low-rank path =================
        with tc.tile_pool(name="sa", bufs=1) as sa, \
             tc.tile_pool(name="pa", bufs=2, space="PSUM") as pa:
            with nc.named_scope("stageA"):
                xs = sa.tile([P, 16, SS], F32)
                nc.sync.dma_start(xs[:], hT_s.rearrange("(kt p) s -> p kt s", p=P))
                wkva = sa.tile([P, 16, 5, P], BF16)
                nc.sync.dma_start(wkva[:], wkv_a_t[:, :, :, :])
                wqa = sa.tile([P, 16, 12, P], BF16)
                nc.sync.dma_start(wqa[:], wq_a_t[:, :, :, :])
                xr = sa.tile([P, 16, SS], BF16)
                nc.vector.tensor_copy(xr[:], xs[:])
                sq = sa.tile([P, 16, SS], BF16)
                nc.vector.tensor_mul(sq[:], xs[:], xs[:])

                # kv-path matmuls on RAW x (RMS commutes; r1 only needed for kpe)
                cvs = sa.tile([P, 5, SS], F32)
                for mc in range(5):
                    ps = pa.tile([P, SS], F32, tag="amm")
                    for kt in range(16):
                        nc.tensor.matmul(ps[:], wkva[:, kt, mc], xr[:, kt],
                                         start=(kt == 0), stop=(kt == 15))
                    nc.vector.tensor_copy(cvs[:, mc], ps[:])

                # r1 (for k_pe only): sum(x^2) -> bcast 64 -> sqrt -> recip
                msq_ps = pa.tile([1, SS], F32, tag="msq")
                for kt in range(16):
                    nc.tensor.matmul(msq_ps[:], ones_r[:], sq[:, kt],
                                     start=(kt == 0), stop=(kt == 15))
                msq_r = sa.tile([1, SS], F32R)
                with nc.allow_low_precision(reason="f32r copy of rms stats"):
                    nc.vector.tensor_copy(msq_r[:], msq_ps[:])
                b1_ps = pa.tile([64, SS], F32, tag="rb")
                nc.tensor.matmul(b1_ps[:], ones_k1[:, :64], msq_r[:],
                                 start=True, stop=True)
                r1s = sa.tile([64, SS], F32)
                nc.scalar.activation(r1s[:], b1_ps[:], AF.Sqrt, scale=1.0 / H,
                                     bias=eps_t[:64])
                r1b = sa.tile([64, SS], F32)
                nc.vector.reciprocal(r1b[:], r1s[:])

                # kv RMS on raw cv (r1 cancels; eps absorbed)
                sq3 = sa.tile([P, 4, SS], BF16)
                nc.vector.tensor_mul(sq3[:], cvs[:, :4], cvs[:, :4])
                msq3 = pa.tile([1, SS], F32, tag="msq")
                for mc in range(4):
                    nc.tensor.matmul(msq3[:], ones_r[:], sq3[:, mc],
                                     start=(mc == 0), stop=(mc == 3))
                msq3_r = sa.tile([1, SS], F32R)
                with nc.allow_low_precision(reason="f32r copy of rms stats"):
                    nc.vector.tensor_copy(msq3_r[:], msq3[:])
                b3_ps = pa.tile([P, SS], F32, tag="rb")
                nc.tensor.matmul(b3_ps[:], ones_k1[:], msq3_r[:],
                                 start=True, stop=True)
                r3s = sa.tile([P, SS], F32)
                nc.scalar.activation(r3s[:], b3_ps[:], AF.Sqrt, scale=1.0 / KVLR,
                                     bias=eps_t[:])
                r3b = sa.tile([P, SS], F32)
                nc.vector.reciprocal(r3b[:], r3s[:])
                ckn = sa.tile([P, 4, SS], BF16)
                nc.vector.tensor_mul(ckn[:], cvs[:, :4],
                                     r3b[:, None, :].to_broadcast([P, 4, SS]))

                # k_pe rope on cvs[:64, 4] * r1 (cs_sh rows 0:64 cos, 64:128 signed sin)
                cos_sh = sa.tile([64, SS], F32)
                nc.sync.dma_start(cos_sh[:], cs_sh[0:64, :])
                sin_sh = sa.tile([64, SS], F32)
                nc.sync.dma_start(sin_sh[:], cs_sh[64:128, :])
                ksw = sa.tile([64, SS], F32)
                nc.sync.dma_start(ksw[0:32, :], cvs[32:64, 4])
                nc.sync.dma_start(ksw[32:64, :], cvs[0:32, 4])
                kpe_c = sa.tile([64, SS], F32)
                nc.vector.tensor_mul(kpe_c[:], cvs[:64, 4], cos_sh[:])
                t1 = sa.tile([64, SS], F32)
                nc.vector.tensor_mul(t1[:], ksw[:], sin_sh[:])
                nc.vector.tensor_add(kpe_c[:], kpe_c[:], t1[:])
                kpe_n = sa.tile([64, SS], BF16)
                nc.vector.tensor_mul(kpe_n[:], kpe_c[:], r1b[:])

                nc.sync.dma_start(
                    ag1a_in[:, 0:4 * SS].rearrange("p (kt s) -> p kt s", s=SS),
                    ckn[:])
                nc.sync.dma_start(ag1a_in[:64, 4 * SS:5 * SS], kpe_n[:])
                nc.gpsimd.collective_compute(
                    "AllGather", mybir.AluOpType.bypass, replica_groups=RG,
                    ins=[ag1a_in], outs=[ag1a_out])

                # q-path on RAW x
                us = sa.tile([P, 12, SS], F32)
                for mc in range(12):
                    ps = pa.tile([P, SS], F32, tag="amm")
                    for kt in range(16):
                        nc.tensor.matmul(ps[:], wqa[:, kt, mc], xr[:, kt],
                                         start=(kt == 0), stop=(kt == 15))
                    nc.vector.tensor_copy(us[:, mc], ps[:])

                sq2 = sa.tile([P, 12, SS], BF16)
                nc.vector.tensor_mul(sq2[:], us[:], us[:])
                msq2 = pa.tile([1, SS], F32, tag="msq")
                for mc in range(12):
                    nc.tensor.matmul(msq2[:], ones_r[:], sq2[:, mc],
                                     start=(mc == 0), stop=(mc == 11))
                msq2_r = sa.tile([1, SS], F32R)
                with nc.allow_low_precision(reason="f32r copy of rms stats"):
                    nc.vector.tensor_copy(msq2_r[:], msq2[:])
                b2_ps = pa.tile([P, SS], F32, tag="rb")
                nc.tensor.matmul(b2_ps[:], ones_k1[:], msq2_r[:],
                                 start=True, stop=True)
                r2s = sa.tile([P, SS], F32)
                nc.scalar.activation(r2s[:], b2_ps[:], AF.Sqrt, scale=1.0 / QLR,
                                     bias=eps_t[:])
                r2b = sa.tile([P, SS], F32)
                nc.vector.reciprocal(r2b[:], r2s[:])
                un = sa.tile([P, 12, SS], BF16)
                nc.vector.tensor_mul(un[:], us[:],
                                     r2b[:, None, :].to_broadcast([P, 12, SS]))
                nc.sync.dma_start(
                    ag1b_in.rearrange("p (kt s) -> p kt s", s=SS), un[:])
                nc.gpsimd.collective_compute(
                    "AllGather", mybir.AluOpType.bypass, replica_groups=RG,
                    ins=[ag1b_in], outs=[ag1b_out])

        # ========== Stage B: qkv + attention + cproj + MLP, interleaved ======
        # Outer pools (whole stage B): h2/residual, cproj tiles, gate/up h0.
        with tc.tile_pool(name="sh2", bufs=1) as sh2, \
             tc.tile_pool(name="scp", bufs=1) as scp, \
             tc.tile_pool(name="sgu", bufs=1) as sgu:

            h2 = sh2.tile([P, 2, S], BF16)
            wos = scp.tile([P, 16, 2, P], BF16)
            nc.sync.dma_start(wos[:], wo_t[:, :, :, :])
            wg0 = sgu.tile([P, 16, 4, P], BF16, name="wg0")
            wu0 = sgu.tile([P, 16, 4, P], BF16, name="wu0")
            nc.sync.dma_start(wg0[:], wg_t[:, 0])
            nc.sync.dma_start(wu0[:], wu_t[:, 0])

            def cproj_a(j, pp, t1, t2, tm):
                """o_proj chunk j + residual + sq-stats + AR4_j."""
                nsl = slice(j * 512, (j + 1) * 512)
                resid = scp.tile([P, 2, 512], F32, tag="resid", bufs=1,
                                 name="resid")
                nc.sync.dma_start(
                    resid[:],
                    hT_r.rearrange("(mc p) s -> p mc s", p=P)[:, :, nsl])
                pss = [pp.tile([P, 512], F32, tag=t1, bufs=2, name="omm"),
                       pp.tile([P, 512], F32, tag=t2, bufs=2, name="omm2")]
                for half in range(2):
                    rhs = scp.tile([P, 8, 512], BF16, tag="rhs2", bufs=2,
                                   name="rhs2")
                    nc.sync.dma_start(
                        rhs[:],
                        ag2_out[j][half * 8 * P:(half + 1) * 8 * P, :].rearrange(
                            "(kt p) s -> p kt s", p=P))
                    for mc in range(2):
                        for ktl in range(8):
                            nc.tensor.matmul(
                                pss[mc][:], wos[:, half * 8 + ktl, mc],
                                rhs[:, ktl],
                                start=(half == 0 and ktl == 0),
                                stop=(half == 1 and ktl == 7))
                sqh = scp.tile([P, 2, 512], BF16, tag="sqh", bufs=1, name="sqh")
                for mc in range(2):
                    nc.vector.tensor_add(h2[:, mc, nsl], pss[mc][:], resid[:, mc])
                    nc.vector.tensor_mul(sqh[:, mc], h2[:, mc, nsl],
                                         h2[:, mc, nsl])
                mt = pp.tile([P, 512], F32, tag=tm, bufs=2 if tm == "dmm" else 1, name="m4")
                for mc in range(2):
                    nc.tensor.matmul(mt[0:1, :], ones_r[:], sqh[:, mc],
                                     start=(mc == 0), stop=(mc == 1))
                msq4 = scp.tile([1, 512], F32, tag="msq4", bufs=1, name="msq4")
                nc.vector.tensor_copy(msq4[:], mt[0:1, :])
                nc.sync.dma_start(ar4_in[j][:, :], msq4[:])
                nc.gpsimd.collective_compute(
                    "AllReduce", mybir.AluOpType.add, replica_groups=RG,
                    ins=[ar4_in[j]], outs=[ar4_out[j]])

            def cproj_b(j, pp, tm):
                """r4_j + yT_j + AG3_j."""
                nsl = slice(j * 512, (j + 1) * 512)
                msq4g = scp.tile([1, 512], F32, tag="m4g", bufs=1, name="msq4g")
                nc.sync.dma_start(msq4g[:], ar4_out[j][:, :])
                msq4r = scp.tile([1, 512], F32R, tag="m4r", bufs=1, name="msq4r")
                with nc.allow_low_precision(reason="f32r copy of rms stats"):
                    nc.vector.tensor_copy(msq4r[:], msq4g[:])
                b4_ps = pp.tile([P, 512], F32, tag=tm, bufs=2 if tm == "dmm" else 1, name="b4")
                nc.tensor.matmul(b4_ps[:], ones_k1[:], msq4r[:],
                                 start=True, stop=True)
                r4s = scp.tile([P, 512], F32, tag="r4s", bufs=1, name="r4s")
                nc.scalar.activation(r4s[:], b4_ps[:], AF.Sqrt,
                                     scale=1.0 / H, bias=eps_t[:])
                r4b = scp.tile([P, 512], F32, tag="r4b", bufs=1, name="r4b")
                nc.vector.reciprocal(r4b[:], r4s[:])
                yT = scp.tile([P, 2, 512], BF16, tag="yT", bufs=1, name="yT")
                nc.vector.tensor_mul(
                    yT[:], h2[:, :, nsl],
                    r4b[:, None, :].to_broadcast([P, 2, 512]))
                nc.sync.dma_start(
                    ag3_in[j].rearrange("(mc p) s -> p mc s", p=P), yT[:])
                nc.gpsimd.collective_compute(
                    "AllGather", mybir.AluOpType.bypass, replica_groups=RG,
                    ins=[ag3_in[j]], outs=[ag3_out[j]])

            # ---- inner phase: qkv + attention (pools released before tail) --
            with tc.tile_pool(name="satt", bufs=1) as satt, \
                 tc.tile_pool(name="qph", bufs=1) as qph, \
                 tc.tile_pool(name="sbr", bufs=2) as sbr, \
                 tc.tile_pool(name="sbe", bufs=2) as sbe, \
                 tc.tile_pool(name="pq", bufs=1, space="PSUM") as pq, \
                 tc.tile_pool(name="pat", bufs=1, space="PSUM") as pat, \
                 tc.tile_pool(name="pcp", bufs=1, space="PSUM") as pcp:

                qT = satt.tile([P, 2, S], BF16)
                kT = satt.tile([P, 2, S], BF16)
                kpeT = satt.tile([64, S], BF16)
                v_tok = satt.tile([P, 32 * P], BF16)
                qrot = [satt.tile([64, 2, 512], BF16, name=f"qrot{i}")
                        for i in range(4)]
                es_all = satt.tile([P, 16, 512], BF16)
                dacc = satt.tile([P, 512], F32R)
                mask_t = satt.tile([P, 4, 512], BF16)
                nc.sync.dma_start(mask_t[:], dmask[:, :, :])
                cos_t = qph.tile([P, S], BF16)
                nc.sync.dma_start(cos_t[:], cossin[0:P, :])
                sin_t = qph.tile([P, S], BF16)
                nc.sync.dma_start(sin_t[:], cossin[P:2 * P, :])
                wkb = qph.tile([P, 4, 4, P], BF16)
                nc.sync.dma_start(wkb[:], wkv_b_t[:, :, :, :])
                wqb = qph.tile([P, 12, 3, P], BF16)
                nc.sync.dma_start(wqb[:], wq_b_t[:, :, :, :])

                def attn(qc):
                    """Attention for query chunk qc, both heads; oT + AG2."""
                    qsl = slice(qc * 512, (qc + 1) * 512)
                    nkt = 4 * qc + 4
                    oT = scp.tile([P, 2, 512], BF16, tag="oT", bufs=1,
                                  name="oT")
                    for h in range(2):
                        for kt in range(nkt):
                            ksl = slice(kt * P, (kt + 1) * P)
                            sc_ps = pat.tile([P, 512], F32, tag="sc", bufs=2,
                                             name="scp")
                            nc.tensor.matmul(sc_ps[:], kT[:, h, ksl],
                                             qT[:, h, qsl],
                                             start=True, stop=False)
                            nc.tensor.matmul(sc_ps[:], kpeT[:, ksl],
                                             qrot[qc][:, h, :],
                                             start=False, stop=True)
                            j = kt - 4 * qc
                            if j >= 0:
                                nc.vector.tensor_add(sc_ps[:], sc_ps[:],
                                                     mask_t[:, j])
                            nc.scalar.activation(es_all[:, kt], sc_ps[:], AF.Exp)
                            with nc.allow_low_precision(reason="f32r denom"):
                                if kt == 0:
                                    nc.vector.tensor_copy(dacc[:],
                                                          es_all[:, kt])
                                else:
                                    nc.vector.tensor_add(dacc[:], dacc[:],
                                                         es_all[:, kt])
                        o_ps = pat.tile([P, 512], F32, tag="o", bufs=1,
                                        name="o")
                        for kt in range(nkt):
                            nc.tensor.matmul(
                                o_ps[:],
                                v_tok[:, (h * 16 + kt) * P:(h * 16 + kt + 1) * P],
                                es_all[:, kt],
                                start=(kt == 0), stop=(kt == nkt - 1))
                        rb_ps = pcp.tile([P, 512], F32, tag="misc", bufs=1,
                                         name="rbo")
                        nc.tensor.matmul(rb_ps[:], ones_pp[:], dacc[:],
                                         start=True, stop=True)
                        recb = sbe.tile([P, 512], F32, tag="recb", bufs=1)
                        nc.vector.reciprocal(recb[:], rb_ps[:])
                        nc.vector.tensor_mul(oT[:, h], o_ps[:], recb[:])
                    nc.sync.dma_start(
                        ag2_in[qc].rearrange("(mc p) s -> p mc s", p=P), oT[:])
                    nc.gpsimd.collective_compute(
                        "AllGather", mybir.AluOpType.bypass, replica_groups=RG,
                        ins=[ag2_in[qc]], outs=[ag2_out[qc]])

                # ---- kv expansion over 4 block-pairs (512 cols each) ----
                with nc.named_scope("stageB_kv"):
                    for bp in range(4):
                        sl = slice(bp * 512, (bp + 1) * 512)
                        rhs_c = sbr.tile([P, 4, 512], BF16, tag="rhs1c",
                                         bufs=2)
                        for half in range(2):
                            blk = 2 * bp + half
                            nc.sync.dma_start(
                                rhs_c[:, :, half * SS:(half + 1) * SS],
                                ag1a_out[blk * P:(blk + 1) * P,
                                         0:4 * SS].rearrange(
                                    "p (kt s) -> p kt s", s=SS))
                            nc.sync.dma_start(
                                kpeT[:, blk * SS:(blk + 1) * SS],
                                ag1a_out[blk * P:blk * P + 64, 4 * SS:5 * SS])
                        for mc in range(2):
                            ps = pq.tile([P, 512], F32, tag="qb", bufs=2,
                                         name="qbp")
                            for kt in range(4):
                                nc.tensor.matmul(ps[:], wkb[:, kt, mc],
                                                 rhs_c[:, kt],
                                                 start=(kt == 0),
                                                 stop=(kt == 3))
                            nc.vector.tensor_copy(kT[:, mc, sl], ps[:])
                        for tt in range(4):
                            tsl = slice(tt * P, (tt + 1) * P)
                            vps = pq.tile([P, 512], F32, tag="qb", bufs=2,
                                          name="qbp")
                            for kt in range(4):
                                nc.tensor.matmul(vps[:, 0:2 * P],
                                                 rhs_c[:, kt, tsl],
                                                 wkb[:, kt, 2:4, :],
                                                 start=(kt == 0),
                                                 stop=(kt == 3))
                            ttg = 4 * bp + tt
                            nc.vector.tensor_copy(
                                v_tok[:, ttg * P:(ttg + 1) * P], vps[:, 0:P])
                            nc.vector.tensor_copy(
                                v_tok[:, (16 + ttg) * P:(17 + ttg) * P],
                                vps[:, P:2 * P])

                # ---- q expansion + rope per block-pair, attn interleaved ----
                with nc.named_scope("stageB_qa"):
                    for bp in range(4):
                        sl = slice(bp * 512, (bp + 1) * 512)
                        rhs_u = sbr.tile([P, 12, 512], BF16, tag="rhs1",
                                         bufs=2)
                        for half in range(2):
                            blk = 2 * bp + half
                            nc.sync.dma_start(
                                rhs_u[:, :, half * SS:(half + 1) * SS],
                                ag1b_out[blk * P:(blk + 1) * P, :].rearrange(
                                    "p (kt s) -> p kt s", s=SS))
                        for mc in range(2):
                            ps = pq.tile([P, 512], F32, tag="qb", bufs=2,
                                         name="qbp")
                            for kt in range(12):
                                nc.tensor.matmul(ps[:], wqb[:, kt, mc],
                                                 rhs_u[:, kt],
                                                 start=(kt == 0),
                                                 stop=(kt == 11))
                            nc.vector.tensor_copy(qT[:, mc, sl], ps[:])
                        qpr_ps = pq.tile([P, 512], F32, tag="qb", bufs=2,
                                         name="qbp")
                        for kt in range(12):
                            nc.tensor.matmul(qpr_ps[:], wqb[:, kt, 2],
                                             rhs_u[:, kt],
                                             start=(kt == 0), stop=(kt == 11))
                        qpr = sbe.tile([P, 512], BF16, tag="qpr", bufs=1)
                        nc.vector.tensor_copy(qpr[:], qpr_ps[:])
                        qsw = sbe.tile([P, 512], BF16, tag="qsw", bufs=1)
                        for b in (0, 64):
                            nc.sync.dma_start(qsw[b:b + 32, :],
                                              qpr[b + 32:b + 64, :])
                            nc.sync.dma_start(qsw[b + 32:b + 64, :],
                                              qpr[b:b + 32, :])
                        qc1 = sbe.tile([P, 512], BF16, tag="qc1", bufs=1)
                        nc.vector.tensor_mul(qc1[:], qpr[:], cos_t[:, sl])
                        qs1 = sbe.tile([P, 512], BF16, tag="qs1", bufs=1)
                        nc.vector.tensor_mul(qs1[:], qsw[:], sin_t[:, sl])
                        qro = sbe.tile([P, 512], BF16, tag="qro", bufs=1)
                        nc.vector.tensor_add(qro[:], qc1[:], qs1[:])
                        nc.sync.dma_start(qrot[bp][:, 0, :], qro[0:64, :])
                        nc.sync.dma_start(qrot[bp][:, 1, :], qro[64:128, :])

                        with nc.named_scope("stageB_attn"):
                            attn(bp)
                            if bp >= 1:
                                cproj_a(bp - 1, pcp, "omm", "omm", "misc")
                            if bp >= 2:
                                cproj_b(bp - 2, pcp, "misc")

            # ---- tail: remaining cproj + MLP fully interleaved ----
            with tc.tile_pool(name="smlp", bufs=1) as smlp, \
                 tc.tile_pool(name="pml", bufs=1, space="PSUM") as pml:
                with nc.named_scope("stageD"):
                    wg1 = smlp.tile([P, 16, 4, P], BF16, name="wg1")
                    wu1 = smlp.tile([P, 16, 4, P], BF16, name="wu1")
                    nc.sync.dma_start(wg1[:], wg_t[:, 1])
                    nc.sync.dma_start(wu1[:], wu_t[:, 1])
                    wds = smlp.tile([P, 8, 16, P], BF16, name="wds")
                    nc.sync.dma_start(wds[:], wd_t[:, :, :, :])

                    acts = {}

                    def gu(half, ncol, wg_s, wu_s):
                        """gate/up for weight half `half`, seq chunk ncol."""
                        nsl = slice(ncol * 512, (ncol + 1) * 512)
                        if ncol not in acts:
                            acts[ncol] = smlp.tile([P, 8, 512], BF16,
                                                   tag="act", bufs=2,
                                                   name=f"act{ncol}")
                        atile = acts[ncol]
                        rhs = smlp.tile([P, 16, 512], BF16, tag="rhs3", bufs=2)
                        nc.sync.dma_start(
                            rhs[:],
                            ag3_out[ncol].rearrange("(kt p) s -> p kt s", p=P))
                        for m in range(4):
                            gp = pml.tile([P, 512], F32, tag="g", bufs=2,
                                          name="gps")
                            up = pml.tile([P, 512], F32, tag="u", bufs=2,
                                          name="ups")
                            for kt in range(16):
                                nc.tensor.matmul(gp[:], wg_s[:, kt, m],
                                                 rhs[:, kt],
                                                 start=(kt == 0),
                                                 stop=(kt == 15))
                                nc.tensor.matmul(up[:], wu_s[:, kt, m],
                                                 rhs[:, kt],
                                                 start=(kt == 0),
                                                 stop=(kt == 15))
                            gsil = smlp.tile([P, 512], BF16, tag="gsil",
                                             bufs=1)
                            nc.scalar.activation(gsil[:], gp[:], AF.Silu)
                            nc.vector.tensor_mul(atile[:, half * 4 + m, :],
                                                 gsil[:], up[:])

                    CH = [(0, 512), (512, 512), (1024, 512), (1536, 256),
                          (1792, 256)]

                    def down(j):
                        """down-proj partials for col chunk j + ReduceScatter."""
                        c0, cw = CH[j]
                        ncol = c0 // 512
                        off = c0 - ncol * 512
                        atile = acts[ncol]
                        for mc in range(16):
                            ps = pml.tile([P, 512], F32, tag="dmm", bufs=2,
                                          name="dmmps")[:, :cw]
                            for kt in range(8):
                                nc.tensor.matmul(
                                    ps[:], wds[:, kt, mc],
                                    atile[:, kt, off:off + cw],
                                    start=(kt == 0), stop=(kt == 7))
                            dn = smlp.tile([P, 512], BF16, tag="dn", bufs=2,
                                           name="dntile")[:, :cw]
                            if mc % 2 == 0:
                                nc.vector.tensor_copy(dn[:], ps[:])
                            else:
                                nc.scalar.activation(dn[:], ps[:], AF.Copy)
                            nc.sync.dma_start(rs_in[j][mc * P:(mc + 1) * P, :],
                                              dn[:])
                        nc.gpsimd.collective_compute(
                            "ReduceScatter", mybir.AluOpType.add,
                            replica_groups=RG, ins=[rs_in[j]], outs=[rs_out[j]])

                    def fin(j):
                        """RS_j output + residual -> outT columns."""
                        c0, cw = CH[j]
                        nsl = slice(c0, c0 + cw)
                        ft = smlp.tile([P, 2, 512], BF16, tag="fin", bufs=1,
                                       name="fintile")[:, :, :cw]
                        nc.sync.dma_start(
                            ft[:],
                            rs_out[j].rearrange("(mc p) s -> p mc s", p=P))
                        fo = smlp.tile([P, 2, 512], F32, tag="fino", bufs=1,
                                       name="finotile")[:, :, :cw]
                        nc.vector.tensor_add(fo[:], ft[:], h2[:, :, nsl])
                        nc.sync.dma_start(
                            outT.rearrange("(mc p) s -> p mc s", p=P)[:, :, nsl],
                            fo[:])

                    gu(0, 0, wg0, wu0)
                    cproj_a(3, pml, "g", "u", "dmm")
                    gu(0, 1, wg0, wu0)
                    cproj_b(2, pml, "dmm")
                    gu(1, 0, wg1, wu1)
                    down(0)
                    gu(0, 2, wg0, wu0)
                    cproj_b(3, pml, "dmm")
                    gu(1, 1, wg1, wu1)
                    down(1)
                    fin(0)
                    gu(0, 3, wg0, wu0)
                    gu(1, 2, wg1, wu1)
                    down(2)
                    fin(1)
                    gu(1, 3, wg1, wu1)
                    down(3)
                    fin(2)
                    down(4)
                    fin(3)
                    fin(4)

    nc.compile()
    _CACHE["nc"] = nc
    return nc


def _host_prep(inputs):
    import ml_dtypes
    bf16 = ml_dtypes.bfloat16

    def _pm(t):
        # [A, M, P, P] tile blocks -> partition-major [P, A, M, P]
        return np.ascontiguousarray(t.transpose(2, 0, 1, 3)).astype(bf16)

    def _pmh(t):
        # [16, 8, P, P] -> [P, half, 16, 4, P]
        t = t.reshape(16, 2, 4, P, P)
        return np.ascontiguousarray(t.transpose(3, 1, 0, 2, 4)).astype(bf16)
    inp = {k: np.asarray(v) for k, v in inputs.items()}
    hidden = inp["hidden_states"].reshape(S, H).astype(np.float32)
    pos = inp["position_ids"].reshape(S).astype(np.int64)
    cosT = inp["cos"][pos].T.astype(np.float32)
    sinT = inp["sin"][pos].T.astype(np.float32)
    wq_a = (inp["wq_a"] * inp["in_ln"][:, None]).astype(np.float32)
    wkv_a = (inp["wkv_a"] * inp["in_ln"][:, None]).astype(np.float32)
    wq_b = (inp["wq_b"] * inp["q_a_ln"][:, None]).astype(np.float32)
    wkv_b = (inp["wkv_b"] * inp["kv_a_ln"][:, None]).astype(np.float32)
    wg = (inp["w_gate"] * inp["post_ln"][:, None]).astype(np.float32)
    wu = (inp["w_up"] * inp["post_ln"][:, None]).astype(np.float32)
    wd = inp["w_down"].astype(np.float32)
    wo = inp["wo"].astype(np.float32)

    de = np.empty(ROPE, np.int64)
    de[:32] = np.arange(32) * 2
    de[32:] = np.arange(32) * 2 + 1
    wkv_a = np.concatenate([wkv_a[:, :KVLR], wkv_a[:, KVLR:][:, de]], axis=1)
    wq_b = wq_b.reshape(QLR, NH, QHD)
    wkv_b = wkv_b.reshape(KVLR, NH, NOPE + VHD)

    hT = hidden.T.copy()
    sin_sg = np.concatenate([-sinT[:32], sinT[32:]], axis=0)    # signed for swap trick
    cossin = np.concatenate([cosT, cosT, sin_sg, sin_sg], axis=0)  # (256, S)
    ki = np.arange(P)[:, None]
    qi = np.arange(512)[None, :]
    dmask = np.stack([np.where(qi >= j * P + ki, 0.0, -1e30).astype(np.float32)
                      for j in range(4)], axis=1)               # (128, 4, 512)

    wq_a_t = _tile_w(wq_a)
    wkv_a_t = _tile_w(wkv_a)

    in_maps = []
    for c in range(NC):
        h0, h1 = 2 * c, 2 * c + 1
        qb = np.concatenate([
            wq_b[:, h0, :NOPE], wq_b[:, h1, :NOPE],
            wq_b[:, h0, NOPE:][:, de], wq_b[:, h1, NOPE:][:, de]], axis=1) * SCALE
        kb = np.concatenate([
            wkv_b[:, h0, :NOPE], wkv_b[:, h1, :NOPE],
            wkv_b[:, h0, NOPE:], wkv_b[:, h1, NOPE:]], axis=1)
        ssl = slice(c * SS, (c + 1) * SS)
        cs_sh = np.concatenate([cosT[:, ssl], sin_sg[:, ssl]], axis=0)
        in_maps.append({
            "hT_s": np.ascontiguousarray(hT[:, ssl]),
            "hT_r": np.ascontiguousarray(hT[ssl, :]),
            "wq_a_t": _pm(wq_a_t),
            "wkv_a_t": _pm(wkv_a_t),
            "wq_b_t": _pm(_tile_w(qb.astype(np.float32))),
            "wkv_b_t": _pm(_tile_w(kb.astype(np.float32))),
            "wo_t": _pm(_tile_w(np.ascontiguousarray(wo[:, ssl]))),
            "wg_t": _pmh(_tile_w(wg[:, c * FFS:(c + 1) * FFS])),
            "wu_t": _pmh(_tile_w(wu[:, c * FFS:(c + 1) * FFS])),
            "wd_t": _pm(_tile_w(wd[c * FFS:(c + 1) * FFS, :])),
            "cossin": cossin.astype(bf16),
            "cs_sh": np.ascontiguousarray(cs_sh),
            "dmask": dmask.astype(bf16),
        })
    return in_maps


_LAST_RESULT = {}


def kernel(**inputs) -> np.ndarray:
    from concourse.bass_utils import run_bass_kernel_spmd
    nc = _build()
    in_maps = _host_prep(inputs)
    kwargs = {}
    if TRACE:
        import sys, types
        if "antenv.axon_hooks" not in sys.modules:
            try:
                from trn_agent_boot.trn_boot import _ntff_profile_via_ctypes
                mod = types.ModuleType("antenv.axon_hooks")
                _hook = _ntff_profile_via_ctypes('/opt/axon/libaxon_pjrt.so')
                mod.get_axon_ntff_profile_hook = lambda: _hook
                mod.set_axon_ntff_profile_hook = lambda h: None
                sys.modules["antenv.axon_hooks"] = mod
                import antenv
                antenv.axon_hooks = mod
            except Exception:
                pass
        kwargs["trace"] = True
    res = run_bass_kernel_spmd(nc, in_maps, list(range(NC)), **kwargs)
    _LAST_RESULT["res"] = res
    outT = np.concatenate([res.results[c]["outT"] for c in range(NC)], axis=0)
    return np.ascontiguousarray(outT.T)[None].astype(np.float32)
